# revision 1
# baseline (speedup 1.0000x reference)
"""Trainium2 Bass kernel for nn_Expert_layer2 (dense per-sample HWxHW attention block).

Sharding: 8 cores = 4 samples x 2 query-halves. Each core receives its sample's
inputs in *band order* (band 0 = the core's query-half rows, band 1 = the other
half), computes the conv/GN/LN pre-projections and the attention for its 2048
queries over all 4096 keys, pair-AllGathers the normalized attention output,
then runs the conv head redundantly in global row order. Host takes pred/xmin
from the even core of each pair.
"""

from contextlib import ExitStack

import numpy as np
import concourse.bass as bass
from concourse import bacc
import concourse.tile as tile
import concourse.mybir as mybir
import concourse.bass_isa as bass_isa
from concourse.bass_utils import run_bass_kernel_spmd

F32 = mybir.dt.float32
F32R = mybir.dt.float32r
AF = mybir.ActivationFunctionType
ALU = mybir.AluOpType

B, C1, K, H, W = 4, 64, 80, 64, 64
HW = H * W
HALF = HW // 2
EPS = 1e-5
GATE_SCALE = 0.1
ISQK = float(1.0 / np.sqrt(np.float32(K)))

_BUILT = None


def _build():
    nc = bacc.Bacc("TRN2", target_bir_lowering=False, num_devices=8)

    dd = {}
    dd["x1b"] = nc.dram_tensor("x1b", [128, 2, 34, 66], F32, kind="ExternalInput")
    dd["x2b"] = nc.dram_tensor("x2b", [128, 2, 34, 66], F32, kind="ExternalInput")
    dd["wpack"] = nc.dram_tensor("wpack", [128, 2471], F32, kind="ExternalInput")
    dd["onesrow"] = nc.dram_tensor("onesrow", [4096], F32, kind="ExternalInput")
    dd["pred_o"] = nc.dram_tensor("pred", [4096], F32, kind="ExternalOutput")
    dd["xmin_o"] = nc.dram_tensor("xminv", [4096], F32, kind="ExternalOutput")
    dd["cc_in"] = nc.dram_tensor("cc_in", [4, 80, 512], F32)
    dd["cc_out"] = nc.dram_tensor("cc_out", [4, 2, 80, 512], F32)

    with tile.TileContext(nc) as tc:
        _body(nc, tc, dd)
    nc.finalize()
    return nc


def _body(nc, tc, dd):
    onesrow = dd["onesrow"]
    ctx = ExitStack()
    with ctx:
        pw = ctx.enter_context(tc.tile_pool(name="pw", bufs=1))
        prow = ctx.enter_context(tc.tile_pool(name="prow", bufs=1))
        pscr = ctx.enter_context(tc.tile_pool(name="pscr", bufs=2))
        pror = ctx.enter_context(tc.tile_pool(name="pror", bufs=1))
        # PSUM: one pool; tags: pstat(x2) cps(x2) scps(x2) pvps(x2) = 8 banks
        psm = ctx.enter_context(tc.tile_pool(name="psm", bufs=2, space="PSUM"))

        # ------------ persistent weights (one packed DMA) ------------
        wpk = pw.tile([128, 2471], F32R, tag="wpk")
        nc.sync.dma_start(out=wpk[:, 0:720], in_=dd["wpack"][:, 0:720].bitcast(F32R))
        nc.gpsimd.dma_start(out=wpk[:, 720:1800], in_=dd["wpack"][:, 720:1800].bitcast(F32R))
        nc.sync.dma_start(out=wpk[:, 1800:2471], in_=dd["wpack"][:, 1800:2471].bitcast(F32R))
        off = [0]

        def _wslice(n):
            a = wpk[:, off[0]:off[0] + n]
            off[0] += n
            return a

        wt1 = _wslice(720).rearrange("p (t f) -> p t f", t=9)
        wtg = _wslice(360).rearrange("p (t f) -> p t f", t=9)
        wth = _wslice(720).rearrange("p (t f) -> p t f", t=9)
        ws = {}
        for nm, fd in (("wid", 80), ("wg2", 80), ("wh2", 1), ("lhsq", 80), ("lhsk", 80),
                       ("lhsv", 80), ("indic", 5), ("indicT", 80), ("indicg", 5),
                       ("indicgT", 40)):
            ws[nm] = _wslice(fd)
        vec_sb = _wslice(12).bitcast(F32)
        ident_sb = _wslice(128).bitcast(F32)
        assert off[0] == 2471
        ones_col = pw.tile([128, 1], F32R, tag="onescol")
        nc.sync.dma_start(out=ones_col[:], in_=onesrow[0:128, None].bitcast(F32R))
        eps_col = pw.tile([128, 1], F32, tag="epscol")
        nc.vector.memset(eps_col[:], EPS)

        gn1_g, gn1_b = vec_sb[0:80, 0:1], vec_sb[0:80, 1:2]
        gnid_g, gnid_b = vec_sb[0:80, 2:3], vec_sb[0:80, 3:4]
        gng_g, gng_b = vec_sb[0:40, 4:5], vec_sb[0:40, 5:6]
        gnh_g, gnh_b = vec_sb[0:80, 6:7], vec_sb[0:80, 7:8]
        lno_g, lno_b = vec_sb[0:80, 8:9], vec_sb[0:80, 9:10]
        bg2_ap = vec_sb[0:80, 10:11]
        bh2_ap = vec_sb[0:1, 11:12]

        # ============ helpers ============
        def token_stats(nm, ntok, chunk_fn, neg_mr, mr_target_ap):
            """chunk_fn(c) -> f32r AP [128, 8, 64] of 512 tokens (pad rows zero).
            Returns (r_row, mr_row) SBUF [1, ntok] tiles (mr_row None if
            mr_target_ap given, in which case the r*m row is DMA'd there)."""
            nchunk = ntok // 512
            dn = nc.dram_tensor(f"dn_{nm}", [2, ntok], F32)
            up = nc.dram_tensor(f"up_{nm}", [2, ntok], F32)
            for c in range(nchunk):
                src = chunk_fn(c)
                sqs = pscr.tile([1, 1024], F32, tag="csq")
                ps_s = psm.tile([1, 512], F32, tag="pstat")
                nc.tensor.matmul(ps_s[:], ones_col[:], src, start=True, stop=True)
                nc.vector.tensor_copy(sqs[:, 0:512], ps_s[:])
                sq = pscr.tile([128, 8, 64], F32R, tag="stq")
                nc.scalar.activation(sq[:], src.bitcast(F32), AF.Square)
                ps_q = psm.tile([1, 512], F32, tag="pstat")
                nc.tensor.matmul(ps_q[:], ones_col[:], sq[:], start=True, stop=True)
                nc.scalar.copy(sqs[:, 512:1024], ps_q[:])
                nc.sync.dma_start(out=dn[:, c * 512:(c + 1) * 512], in_=sqs[:])
            ncol = ntok // 128
            st = prow.tile([128, 2, ncol], F32, tag=f"stt_{nm}")
            nc.sync.dma_start(out=st[:], in_=dn.rearrange("r (p c) -> p r c", p=128))
            m = prow.tile([128, ncol], F32, tag=f"m_{nm}")
            nc.vector.tensor_scalar(m[:], st[:, 0, :], 1.0 / K, None, ALU.mult)
            var = prow.tile([128, ncol], F32, tag=f"v_{nm}")
            nc.vector.tensor_scalar(var[:], st[:, 1, :], 1.0 / K, None, ALU.mult)
            msq = prow.tile([128, ncol], F32, tag=f"ms_{nm}")
            nc.vector.tensor_tensor(msq[:], m[:], m[:], ALU.mult)
            nc.vector.tensor_tensor(var[:], var[:], msq[:], ALU.subtract)
            nc.scalar.activation(var[:], var[:], AF.Sqrt, bias=eps_col[:])
            rup = prow.tile([128, 2, ncol], F32, tag=f"ru_{nm}")
            nc.vector.reciprocal(rup[:, 0, :], var[:])
            nc.vector.tensor_tensor(rup[:, 1, :], rup[:, 0, :], m[:], ALU.mult)
            if neg_mr:
                nc.vector.tensor_scalar(rup[:, 1, :], rup[:, 1, :], -1.0, None, ALU.mult)
            nc.sync.dma_start(out=up.rearrange("r (p c) -> p r c", p=128), in_=rup[:])
            r_row = pror.tile([1, 4096], F32, tag="rrow", name="r_row")[:, 0:ntok]
            nc.sync.dma_start(out=r_row, in_=up[0, None, :])
            if mr_target_ap is not None:
                nc.sync.dma_start(out=mr_target_ap, in_=up[1, None, :].bitcast(F32R))
                return r_row, None
            mr_row = pror.tile([1, 4096], F32, tag="rrow", name="mr_row")[:, 0:ntok]
            nc.sync.dma_start(out=mr_row, in_=up[1, None, :])
            return r_row, mr_row

        def conv3x3(wt_sb, nout, win_fn, out_sb, acc, accsq):
            """win_fn(c8, dy, dx) -> rhs AP [128, 8, 64]. out_sb: [128, 4096] f32r."""
            for c8 in range(8):
                ps = psm.tile([nout, 512], F32, tag="cps")
                for tap in range(9):
                    dy, dx = divmod(tap, 3)
                    nc.tensor.matmul(ps[:], wt_sb[:, tap, :], win_fn(c8, dy, dx),
                                     start=(tap == 0), stop=(tap == 8))
                nc.scalar.activation(out_sb[0:nout, c8 * 512:(c8 + 1) * 512], ps[:],
                                     AF.Copy, accum_out=acc[:, c8:c8 + 1])
                sq = pscr.tile([128, 512], F32, tag="csq")
                nc.scalar.activation(sq[0:nout, :], ps[:], AF.Square,
                                     accum_out=accsq[:, c8:c8 + 1])

        def gn_params(nm, acc, accsq, nch, ind_sb, indT_sb, gamma, beta, inv_n):
            s_ch = prow.tile([nch, 2], F32, tag=f"sch_{nm}")
            nc.vector.tensor_reduce(s_ch[:, 0:1], acc[:], mybir.AxisListType.X, ALU.add)
            nc.vector.tensor_reduce(s_ch[:, 1:2], accsq[:], mybir.AxisListType.X, ALU.add)
            chs = pscr.tile([128, 2], F32R, tag="chs")
            nc.vector.memset(chs[:].bitcast(F32), 0.0)
            nc.vector.tensor_copy(chs[0:nch, :], s_ch[:])
            ps_g = psm.tile([5, 2], F32, tag="pstat")
            nc.tensor.matmul(ps_g[:], ind_sb, chs[:], start=True, stop=True)
            gst = pscr.tile([128, 2], F32R, tag="gst")
            nc.vector.memset(gst[:].bitcast(F32), 0.0)
            nc.vector.tensor_scalar(gst[0:5, 0:1], ps_g[:, 0:1], inv_n, None, ALU.mult)
            e2 = prow.tile([5, 1], F32, tag=f"e2_{nm}")
            nc.vector.tensor_scalar(e2[:], ps_g[:, 1:2], inv_n, None, ALU.mult)
            vr = prow.tile([5, 1], F32, tag=f"vr_{nm}")
            nc.vector.tensor_tensor(vr[:], gst[0:5, 0:1].bitcast(F32),
                                    gst[0:5, 0:1].bitcast(F32), ALU.mult)
            nc.vector.tensor_tensor(vr[:], e2[:], vr[:], ALU.subtract)
            nc.scalar.activation(vr[:], vr[:], AF.Sqrt, bias=eps_col[0:5, :])
            rst = prow.tile([5, 1], F32, tag=f"rst_{nm}")
            nc.vector.reciprocal(rst[:], vr[:])
            nc.vector.tensor_copy(gst[0:5, 1:2], rst[:])
            ps_bc = psm.tile([nch, 2], F32, tag="pstat")
            nc.tensor.matmul(ps_bc[:], indT_sb, gst[:], start=True, stop=True)
            a = prow.tile([nch, 1], F32, tag=f"a_{nm}")
            c = prow.tile([nch, 1], F32, tag=f"c_{nm}")
            nc.vector.tensor_tensor(a[:], gamma, ps_bc[:, 1:2], ALU.mult)
            nc.vector.tensor_tensor(c[:], ps_bc[:, 0:1], a[:], ALU.mult)
            nc.vector.tensor_tensor(c[:], beta, c[:], ALU.subtract)
            return a, c

        # ============ stage 1a: x1 conv (x1pad dies right after) ============
        p2b = ctx.enter_context(tc.tile_pool(name="p2b", bufs=1))
        ctx_mid = ExitStack()
        pmid = ctx_mid.enter_context(tc.tile_pool(name="pmid", bufs=1))
        with tc.tile_pool(name="pin1", bufs=1) as pin1:
            x1pad = pin1.tile([128, 2, 34, 66], F32R, tag="x1pad")
            nc.sync.dma_start(out=x1pad[:, 0], in_=dd["x1b"][:, 0].bitcast(F32R))
            nc.sync.dma_start(out=x1pad[:, 1], in_=dd["x1b"][:, 1].bitcast(F32R))
            x1cp = pmid.tile([128, 4096], F32R, tag="x1cp")
            nc.vector.memset(x1cp[:].bitcast(F32), 0.0)
            acc1 = prow.tile([80, 8], F32, tag="acc1")
            accsq1 = prow.tile([80, 8], F32, tag="accsq1")

            def x1win(c8, dy, dx):
                j, c4 = divmod(c8, 4)
                return x1pad[:, j, c4 * 8 + dy: c4 * 8 + dy + 8, dx:dx + 64]

            conv3x3(wt1, 80, x1win, x1cp, acc1, accsq1)

        a1, c1 = gn_params("1", acc1, accsq1, 80, ws["indic"], ws["indicT"],
                           gn1_g, gn1_b, 1.0 / (16 * HW))
        # GN + SiLU in place -> x1p (rows 0..79 of x1cp)
        nc.scalar.activation(x1cp[0:80, :], x1cp[0:80, :].bitcast(F32), AF.Silu,
                             bias=c1[:], scale=a1[:])

        # ============ stage 1b: x2 side ============
        pin2 = ctx_mid.enter_context(tc.tile_pool(name="pin2", bufs=1))
        x2pad = pin2.tile([128, 2, 34, 66], F32R, tag="x2pad")
        nc.gpsimd.dma_start(out=x2pad[:, 0], in_=dd["x2b"][:, 0].bitcast(F32R))
        nc.gpsimd.dma_start(out=x2pad[:, 1], in_=dd["x2b"][:, 1].bitcast(F32R))

        def x2win(c8):
            j, c4 = divmod(c8, 4)
            return x2pad[:, j, 1 + c4 * 8: 9 + c4 * 8, 1:65]

        # xmin (band order): four 1024-token passes, exact f32 read of x2
        for cq in range(4):
            j, c2 = divmod(cq, 2)
            xw = pscr.tile([80, 16, 64], F32, tag="csq")
            nc.sync.dma_start(out=xw[:], in_=dd["x2b"][0:80, j, 1 + 16 * c2: 17 + 16 * c2, 1:65])
            ng = pscr.tile([80, 1024], F32, tag="csq")
            nc.vector.tensor_scalar(ng[:].rearrange("p (a b) -> p a b", a=16),
                                    xw[:], -1.0, None, ALU.mult)
            ar = pscr.tile([80, 1024], F32, tag="csq")
            nc.gpsimd.partition_all_reduce(ar[:], ng[:], channels=80,
                                           reduce_op=bass_isa.ReduceOp.max)
            xm = pror.tile([1, 4096], F32, tag="rrow", name="xm")
            nc.vector.tensor_scalar(xm[:, 0:1024], ar[0:1, :], -1.0, None, ALU.mult)
            nc.sync.dma_start(out=dd["xmin_o"][None, cq * 1024:(cq + 1) * 1024],
                              in_=xm[:, 0:1024])

        # x2 token LN stats -> xh2
        xh2 = pmid.tile([128, 4096], F32R, tag="xh2")
        nc.vector.memset(xh2[:].bitcast(F32), 0.0)
        r2_row, _ = token_stats("x2", 4096, x2win, False, xh2[80:81, :])
        nc.sync.dma_start(out=xh2[81:82, :], in_=onesrow[None, :].bitcast(F32R))
        for c in range(8):
            sl = slice(c * 512, (c + 1) * 512)
            bc = pscr.tile([80, 512], F32, tag="bscr", name="bc")
            nc.gpsimd.partition_broadcast(bc[:], r2_row[0:1, sl])
            nc.vector.tensor_tensor(xh2[0:80, sl].rearrange("p (a b) -> p a b", a=8),
                                    x2win(c)[0:80].bitcast(F32),
                                    bc[:].rearrange("p (a b) -> p a b", a=8), ALU.mult)

        # k, v projections (LN folded into lhs weights)
        k_sb = p2b.tile([128, 4096], F32R, tag="k_sb")
        nc.vector.memset(k_sb[:].bitcast(F32), 0.0)
        v_sb = pmid.tile([128, 4096], F32, tag="v_sb")
        nc.vector.memset(v_sb[:], 0.0)
        for c in range(8):
            sl = slice(c * 512, (c + 1) * 512)
            ps_k = psm.tile([80, 512], F32, tag="cps")
            nc.tensor.matmul(ps_k[:], ws["lhsk"], xh2[:, sl], start=True, stop=True)
            nc.vector.tensor_copy(k_sb[0:80, sl], ps_k[:])
            ps_v = psm.tile([80, 512], F32, tag="cps")
            nc.tensor.matmul(ps_v[:], ws["lhsv"], xh2[:, sl], start=True, stop=True)
            nc.vector.tensor_copy(v_sb[0:80, sl], ps_v[:])

        # gate branch: conv3x3 -> GN+SiLU (in place) -> 1x1 -> sigmoid -> gate v
        gcp = pmid.tile([128, 4096], F32R, tag="gcp")
        nc.vector.memset(gcp[:].bitcast(F32), 0.0)
        accg = prow.tile([40, 8], F32, tag="accg")
        accsqg = prow.tile([40, 8], F32, tag="accsqg")

        def gwin(c8, dy, dx):
            j, c4 = divmod(c8, 4)
            return x2pad[:, j, c4 * 8 + dy: c4 * 8 + dy + 8, dx:dx + 64]

        conv3x3(wtg, 40, gwin, gcp, accg, accsqg)
        ag, cg = gn_params("g", accg, accsqg, 40, ws["indicg"], ws["indicgT"],
                           gng_g, gng_b, 1.0 / (8 * HW))
        nc.scalar.activation(gcp[0:40, :], gcp[0:40, :].bitcast(F32), AF.Silu,
                             bias=cg[:], scale=ag[:])
        for c in range(8):
            sl = slice(c * 512, (c + 1) * 512)
            ps_g2 = psm.tile([80, 512], F32, tag="cps")
            nc.tensor.matmul(ps_g2[:], ws["wg2"], gcp[:, sl], start=True, stop=True)
            sg = pscr.tile([80, 512], F32, tag="bscr")
            nc.scalar.activation(sg[:], ps_g2[:], AF.Sigmoid, bias=bg2_ap)
            nc.vector.tensor_scalar(sg[:], sg[:], GATE_SCALE, 1.0, ALU.mult, ALU.add)
            nc.vector.tensor_tensor(v_sb[0:80, sl], v_sb[0:80, sl], sg[:], ALU.mult)

        # x2_id: conv1x1 + GN (store first half only; stats from accums)
        idc = p2b.tile([80, HALF], F32, tag="idc")
        accid = prow.tile([80, 8], F32, tag="accid")
        accsqid = prow.tile([80, 8], F32, tag="accsqid")
        for c in range(8):
            ps_id = psm.tile([80, 512], F32, tag="cps")
            nc.tensor.matmul(ps_id[:], ws["wid"], x2win(c), start=True, stop=True)
            if c < 4:
                out_ap = idc[:, c * 512:(c + 1) * 512]
            else:
                idscr = pscr.tile([80, 512], F32, tag="bscr", name="idscr")
                out_ap = idscr[:]
            nc.scalar.activation(out_ap, ps_id[:], AF.Copy, accum_out=accid[:, c:c + 1])
            sqi = pscr.tile([128, 512], F32, tag="csq")
            nc.scalar.activation(sqi[0:80, :], ps_id[:], AF.Square,
                                 accum_out=accsqid[:, c:c + 1])
        aid, cid = gn_params("id", accid, accsqid, 80, ws["indic"], ws["indicT"],
                             gnid_g, gnid_b, 1.0 / (16 * HW))
        nc.vector.tensor_scalar(idc[:], idc[:], aid[:], cid[:], ALU.mult, ALU.add)

        # vT: token-major gated v in cols 0..79, ones in col 96 (denominator row)
        vT = p2b.tile([128, 32, 97], F32R, tag="vT")
        nc.vector.memset(vT[:].bitcast(F32), 1.0)
        for kc in range(32):
            ps_t = psm.tile([128, 128], F32, tag="cps")
            nc.tensor.transpose(ps_t[:], v_sb[:, kc * 128:(kc + 1) * 128], ident_sb)
            nc.vector.tensor_copy(vT[:, kc, 0:80], ps_t[:, 0:80])

        # x1 token LN stats (first HALF only) -> xh1 -> q
        xh1 = pmid.tile([128, HALF], F32R, tag="xh1")
        nc.vector.memset(xh1[:].bitcast(F32), 0.0)
        r1_row, _ = token_stats(
            "x1", HALF,
            lambda c: x1cp[:, c * 512:(c + 1) * 512].rearrange("p (a b) -> p a b", a=8),
            False, xh1[80:81, :])
        nc.sync.dma_start(out=xh1[81:82, :], in_=onesrow[None, 0:HALF].bitcast(F32R))
        for c in range(4):
            sl = slice(c * 512, (c + 1) * 512)
            bc = pscr.tile([80, 512], F32, tag="bscr", name="bc")
            nc.gpsimd.partition_broadcast(bc[:], r1_row[0:1, sl])
            nc.vector.tensor_tensor(xh1[0:80, sl], x1cp[0:80, sl].bitcast(F32),
                                    bc[:], ALU.mult)
        q_sb = p2b.tile([128, HALF], F32R, tag="q_sb")
        nc.vector.memset(q_sb[:].bitcast(F32), 0.0)
        for c in range(4):
            sl = slice(c * 512, (c + 1) * 512)
            ps_q = psm.tile([80, 512], F32, tag="cps")
            nc.tensor.matmul(ps_q[:], ws["lhsq"], xh1[:, sl], start=True, stop=True)
            nc.vector.tensor_copy(q_sb[0:80, sl], ps_q[:])

        ctx_mid.close()

        # ============ stage 2: attention ============
        yres = p2b.tile([128, HALF], F32R, tag="yres")
        nc.vector.memset(yres[:].bitcast(F32), 0.0)
        for qc in range(4):
            qsl = slice(qc * 512, (qc + 1) * 512)
            ps_y = psm.tile([97, 512], F32, tag="cps")
            for k2 in range(16):
                ps_s = psm.tile([128, 1024], F32, tag="scps", name="ps_s")
                nc.tensor.matmul(ps_s[:, 0:512], k_sb[:, (2 * k2) * 128:(2 * k2 + 1) * 128],
                                 q_sb[:, qsl], start=True, stop=True)
                nc.tensor.matmul(ps_s[:, 512:1024], k_sb[:, (2 * k2 + 1) * 128:(2 * k2 + 2) * 128],
                                 q_sb[:, qsl], start=True, stop=True)
                e_sb = pscr.tile([128, 1024], F32R, tag="e_sb")
                nc.scalar.activation(e_sb[:], ps_s[:], AF.Exp, scale=ISQK)
                nc.tensor.matmul(ps_y[:], vT[:, 2 * k2, :], e_sb[:, 0:512],
                                 start=(k2 == 0), stop=False)
                nc.tensor.matmul(ps_y[:], vT[:, 2 * k2 + 1, :], e_sb[:, 512:1024],
                                 start=False, stop=(k2 == 15))
            rec = pscr.tile([1, 512], F32, tag="bscr")
            nc.vector.reciprocal(rec[:], ps_y[96:97, :])
            rb = pscr.tile([80, 512], F32, tag="bscr")
            nc.gpsimd.partition_broadcast(rb[:], rec[:])
            nc.vector.tensor_tensor(yres[0:80, qsl], ps_y[0:80, :], rb[:], ALU.mult)
            nc.vector.tensor_tensor(yres[0:80, qsl], yres[0:80, qsl].bitcast(F32),
                                    idc[:, qsl], ALU.add)

            # out-LN on this 512-token block: stats stay on one partition
            # (no DRAM bounce), then pairwise gather overlapping the next qc
            src = yres[:, qsl].rearrange("p (a b) -> p a b", a=8)
            ps_s = psm.tile([1, 512], F32, tag="cps", name="ps_s_o")
            nc.tensor.matmul(ps_s[:], ones_col[:], src, start=True, stop=True)
            sqo = pscr.tile([128, 8, 64], F32R, tag="stq", name="sqo")
            nc.scalar.activation(sqo[:], src.bitcast(F32), AF.Square)
            ps_q = psm.tile([1, 512], F32, tag="cps", name="ps_q_o")
            nc.tensor.matmul(ps_q[:], ones_col[:], sqo[:], start=True, stop=True)
            t_m = pscr.tile([1, 512], F32, tag="tm")
            nc.vector.tensor_scalar(t_m[:], ps_s[:], 1.0 / K, None, ALU.mult)
            t_r = pscr.tile([1, 512], F32, tag="tr")
            nc.vector.tensor_scalar(t_r[:], ps_q[:], 1.0 / K, None, ALU.mult)
            t_n = pscr.tile([1, 512], F32, tag="tn")
            nc.vector.tensor_tensor(t_n[:], t_m[:], t_m[:], ALU.mult)
            nc.vector.tensor_tensor(t_r[:], t_r[:], t_n[:], ALU.subtract)
            nc.scalar.activation(t_r[:], t_r[:], AF.Sqrt, bias=eps_col[0:1, :])
            nc.vector.reciprocal(t_r[:], t_r[:])
            nc.vector.tensor_tensor(t_n[:], t_r[:], t_m[:], ALU.mult)
            nc.vector.tensor_scalar(t_n[:], t_n[:], -1.0, None, ALU.mult)
            bc = pscr.tile([80, 512], F32, tag="bscr", name="bc")
            nc.gpsimd.partition_broadcast(bc[:], t_r[:])
            mbc = pscr.tile([80, 512], F32, tag="bscr", name="mbc")
            nc.gpsimd.partition_broadcast(mbc[:], t_n[:])
            yl = pscr.tile([80, 512], F32, tag="csq")
            nc.vector.tensor_tensor(yl[:], yres[0:80, qsl].bitcast(F32), bc[:], ALU.mult)
            nc.vector.tensor_tensor(yl[:], yl[:], mbc[:], ALU.add)
            nc.vector.tensor_scalar(yl[:], yl[:], lno_g, lno_b, ALU.mult, ALU.add)
            nc.sync.dma_start(out=dd["cc_in"][qc], in_=yl[:])
            nc.gpsimd.collective_compute(
                "AllGather", ALU.bypass,
                replica_groups=[[0, 1], [2, 3], [4, 5], [6, 7]],
                ins=[dd["cc_in"][qc][:]], outs=[dd["cc_out"][qc][:]],
            )

        ph = ctx.enter_context(tc.tile_pool(name="ph", bufs=1))
        ypad = ph.tile([128, 66, 66], F32R, tag="ypad")
        nc.vector.memset(ypad[:].bitcast(F32), 0.0)
        for qc in range(4):
            for r in range(2):
                nc.sync.dma_start(
                    out=ypad[0:80, 1 + 32 * r + 8 * qc: 9 + 32 * r + 8 * qc, 1:65],
                    in_=dd["cc_out"][qc, r].rearrange("p (a b) -> p a b", a=8).bitcast(F32R))

        hcp = ph.tile([128, 4096], F32R, tag="hcp")
        nc.vector.memset(hcp[:].bitcast(F32), 0.0)
        acch = prow.tile([80, 8], F32, tag="acch")
        accsqh = prow.tile([80, 8], F32, tag="accsqh")

        def hwin(c8, dy, dx):
            return ypad[:, c8 * 8 + dy: c8 * 8 + dy + 8, dx:dx + 64]

        conv3x3(wth, 80, hwin, hcp, acch, accsqh)
        ah, ch_ = gn_params("h", acch, accsqh, 80, ws["indic"], ws["indicT"],
                            gnh_g, gnh_b, 1.0 / (16 * HW))
        nc.scalar.activation(hcp[0:80, :], hcp[0:80, :].bitcast(F32), AF.Silu,
                             bias=ch_[:], scale=ah[:])
        pr = pror.tile([1, 4096], F32, tag="rrow", name="pr")
        for c in range(8):
            sl = slice(c * 512, (c + 1) * 512)
            ps_p = psm.tile([1, 512], F32, tag="pstat")
            nc.tensor.matmul(ps_p[:], ws["wh2"], hcp[:, sl], start=True, stop=True)
            nc.scalar.activation(pr[:, sl], ps_p[:], AF.Sigmoid, bias=bh2_ap)
        nc.sync.dma_start(out=dd["pred_o"][None, :], in_=pr[:])


def _prep_inputs(x1, x2, w_p1, gn1_g, gn1_b, w_id, gnid_g, gnid_b, wq, wk, wv,
                 wg1, gng_g, gng_b, wg2, bg2, lnx1_g, lnx1_b, lnx2_g, lnx2_b,
                 lno_g, lno_b, wh1, gnh_g, gnh_b, wh2, bh2):
    f = np.float32

    def band_pack(x, C):
        out = {}
        for h in (0, 1):
            arr = np.zeros((128, 2, 34, 66), f)
            for j in range(2):
                hj = h if j == 0 else 1 - h
                g0 = 32 * hj - 1
                lo, hi = max(0, g0), min(64, g0 + 34)
                arr[:C, j, lo - g0: hi - g0, 1:65] = x[:, lo:hi, :]
            out[h] = arr
        return out

    def fold_lhs(wmat, lng, lnb):
        wmat = np.asarray(wmat).astype(f)
        wp = wmat * np.asarray(lng).astype(f)[None, :]
        lhs = np.zeros((128, 80), f)
        lhs[0:80, :] = wp.T
        lhs[80, :] = -wp.sum(axis=1)
        lhs[81, :] = wmat @ np.asarray(lnb).astype(f)
        return lhs

    def taps(wc, cin, cout):
        wt = np.zeros((9, 128, cout), f)
        wc = np.asarray(wc).astype(f)
        for t in range(9):
            dy, dx = divmod(t, 3)
            wt[t, 0:cin, :] = wc[:, :, dy, dx].T
        return wt

    wid_t = np.zeros((128, 80), f); wid_t[0:80, :] = np.asarray(w_id)[:, :, 0, 0].T
    wg2_t = np.zeros((128, 80), f); wg2_t[0:40, :] = np.asarray(wg2)[:, :, 0, 0].T
    wh2_t = np.zeros((128, 1), f); wh2_t[0:80, 0] = np.asarray(wh2)[0, :, 0, 0]
    indic = np.zeros((128, 5), f); indicT = np.zeros((128, 80), f)
    for ch in range(80):
        indic[ch, ch // 16] = 1.0
        indicT[ch // 16, ch] = 1.0
    indicg = np.zeros((128, 5), f); indicgT = np.zeros((128, 40), f)
    for ch in range(40):
        indicg[ch, ch // 8] = 1.0
        indicgT[ch // 8, ch] = 1.0
    vecs = np.zeros((128, 12), f)
    for col, v, n in ((0, gn1_g, 80), (1, gn1_b, 80), (2, gnid_g, 80), (3, gnid_b, 80),
                     (4, gng_g, 40), (5, gng_b, 40), (6, gnh_g, 80), (7, gnh_b, 80),
                     (8, lno_g, 80), (9, lno_b, 80), (10, bg2, 80), (11, bh2, 1)):
        vecs[0:n, col] = np.asarray(v).astype(f)

    wpack = np.concatenate([
        taps(w_p1, C1, 80).transpose(1, 0, 2).reshape(128, 720),
        taps(wg1, 80, 40).transpose(1, 0, 2).reshape(128, 360),
        taps(wh1, 80, 80).transpose(1, 0, 2).reshape(128, 720),
        wid_t, wg2_t, wh2_t,
        fold_lhs(wq, lnx1_g, lnx1_b), fold_lhs(wk, lnx2_g, lnx2_b),
        fold_lhs(wv, lnx2_g, lnx2_b),
        indic, indicT, indicg, indicgT, vecs,
        np.eye(128, dtype=f)], axis=1).astype(f)
    shared = dict(wpack=wpack, onesrow=np.ones(4096, f))
    in_maps = []
    for core in range(8):
        b, h = divmod(core, 2)
        in_maps.append(dict(
            x1b=band_pack(np.asarray(x1)[b].astype(f), C1)[h],
            x2b=band_pack(np.asarray(x2)[b].astype(f), K)[h],
            **shared))
    return in_maps


def kernel(**inputs):
    global _BUILT
    if _BUILT is None:
        _BUILT = _build()
    in_maps = _prep_inputs(**inputs)
    last_err = None
    for _ in range(3):
        try:
            res = run_bass_kernel_spmd(_BUILT, in_maps, list(range(8))).results
            break
        except Exception as e:  # transient axon worker hangups
            last_err = e
    else:
        raise last_err
    pred = np.stack([res[2 * b]["pred"] for b in range(B)]).astype(np.float32)
    xmin = np.stack([res[2 * b]["xminv"] for b in range(B)]).astype(np.float32)
    return pred, xmin

